# revision 1
# baseline (speedup 1.0000x reference)
"""DSS layer kernel for Trainium2 (8 NeuronCores, SPMD, no collectives).

Math: the reference's FFT conv kernel k[h,l] = Re(Wc @ exp(Lam*t)) has
|exp(Lam*t)| = e^{-l/2}, so taps beyond m=64 are < 1e-12 relative -- the conv
is a 65-tap causal FIR. We implement it as overlap-save block convolution:
  - window F=256, hop=192, left halo 64
  - half-shifted real DFT (bins f+1/2, f=0..127): exactly 128 complex bins
    (no DC/Nyquist degeneracy), diagonalizes negacyclic convolution; the
    aliased first 64 outputs of each window are discarded, so linear (causal)
    convolution is exact.
  - all transform matmuls have weights shared across channels (PE-friendly);
    the per-channel part is an elementwise spectrum product (DVE mults in
    bf16, add/sub on GPSIMD), with gelu on ACT.
Sharding: 8 cores = (batch b, L-half). Each core computes all 512 channels for
its 1024 time steps, so the final 512x512 linear needs no cross-core comm.
Host passes per-core u pre-transposed [time, H] (plus a 64-row-shifted view
loaded from the same buffer so odd windows stay 128-partition aligned) and
small constant tables: forward DFT matrices, P-tables folding the Vandermonde
S-table (built from the fp32-rounded phases to match the reference) with the
kernel-side DFT, bf16 inverse-DFT matrices, and lin_w^T in bf16.
Matmuls run in float32r (single-pass fp32, ~1.6e-4 rel) or bf16; a short
warmup matmul stream ramps the PE clock while the first DMAs land.
"""

import numpy as np

H = 512
N = 64
B = 4
L = 2048
K = 65          # FIR taps
F = 256         # DFT window
HOP = 192
HALO = 64
NWIN = 6
LLOC = L // 2   # 1024 per core
ROWS = HALO + NWIN * HOP   # 1216 rows of u^T per core
RPAD = 1280                # padded to 10 * 128
QCOLS = RPAD // 128
HT = H // 128   # 4 h-tiles
NCORES = 8
BLOB_COLS = 2182

_cache = {}


def _build_nc(stage=99):
    import concourse.bacc as bacc
    import concourse.tile as tile
    from concourse import mybir

    f32 = mybir.dt.float32
    f32r = mybir.dt.float32r
    nc = bacc.Bacc(None, target_bir_lowering=False)

    ut = nc.dram_tensor("ut", [RPAD, H], f32, kind="ExternalInput")
    bloba = nc.dram_tensor("bloba", [64, 1536], f32, kind="ExternalInput")
    blobb = nc.dram_tensor("blobb", [128, 516], f32, kind="ExternalInput")
    blobc = nc.dram_tensor("blobc", [128, 512], mybir.dt.uint16, kind="ExternalInput")
    lwt = nc.dram_tensor("lwt", [H, H], mybir.dt.uint16, kind="ExternalInput")
    y2 = nc.dram_tensor("y2", [H, LLOC], f32, kind="ExternalOutput")

    GELU = mybir.ActivationFunctionType.Gelu
    bf16 = mybir.dt.bfloat16

    with tile.TileContext(nc) as tc:
        with (
            tc.tile_pool(name="consts", bufs=1) as consts,
            tc.tile_pool(name="scratch", bufs=3) as scratch,
        ):
            # ---------- loads (kgen-critical first, small, parallel queues) ----------
            # warm tile: cheap memset (no DMA) -- the warmup matmuls only
            # ramp the PE clock, their values are discarded
            warm_sb = consts.tile([128, 512], f32, tag="warm")
            nc.vector.memset(warm_sb, 0.0)
            bloba_sb = consts.tile([64, 1536], f32r, tag="bloba")
            nc.sync.dma_start(out=bloba_sb, in_=bloba[:, :].bitcast(f32r))
            blobb_sb = consts.tile([128, 516], f32r, tag="blobb")
            blobc_sb = consts.tile([128, 2, 256], bf16, tag="blobc")
            nc.sync.dma_start(out=blobc_sb,
                              in_=blobc[:, :].bitcast(bf16)
                              .rearrange("p (a f) -> p a f", a=2))

            wrt_sb = bloba_sb[:, 0:512]
            wit_sb = bloba_sb[:, 512:1024]
            p1_sb = bloba_sb[:, 1024:1152]
            p2_sb = bloba_sb[:, 1152:1280]
            p3_sb = bloba_sb[:, 1280:1408]
            p4_sb = bloba_sb[:, 1408:1536]
            dfc_sb = blobb_sb[:, 0:256].rearrange("p (a f) -> p a f", a=2)
            dfsn_sb = blobb_sb[:, 256:512].rearrange("p (a f) -> p a f", a=2)
            lb_sb = blobb_sb[:, 512:516].bitcast(f32)
            iccb_sb = blobc_sb[:, 0, :]
            icsnb_sb = blobc_sb[:, 1, :]

            # u columns as pair-loads, spread across queues, in consumption
            # order; u2 is the 64-row-shifted copy for odd windows
            u_sb = consts.tile([128, 8, H], f32r, tag="u_sb")
            u2_sb = consts.tile([128, 9, H], f32r, tag="u2_sb")
            nc.scalar.dma_start(out=blobb_sb, in_=blobb[:, :].bitcast(f32r))
            upairs = [("u", 0, nc.gpsimd), ("u2", 1, nc.scalar), ("u", 3, nc.gpsimd),
                      ("u2", 4, nc.gpsimd), ("u", 6, nc.sync), ("u2", 7, nc.sync)]
            for which, q, eng in upairs:
                if which == "u":
                    eng.dma_start(
                        out=u_sb[:, q:q + 2, :],
                        in_=ut[q * 128:(q + 2) * 128, :].bitcast(f32r)
                        .rearrange("(q p) h -> p q h", p=128))
                else:
                    eng.dma_start(
                        out=u2_sb[:, q:q + 2, :],
                        in_=ut[64 + q * 128:64 + (q + 2) * 128, :].bitcast(f32r)
                        .rearrange("(q p) h -> p q h", p=128))
            lwt_sb = consts.tile([128, HT, H], bf16, tag="lwt")
            nc.scalar.dma_start(out=lwt_sb,
                                in_=lwt[:, :].bitcast(bf16)
                                .rearrange("(a p) o -> p a o", p=128))

            krb_sb = consts.tile([128, H], bf16, tag="krb")
            kib_sb = consts.tile([128, H], bf16, tag="kib")
            y1_sb = consts.tile([128, HT, LLOC], bf16, tag="y1")
            y2_sb = [consts.tile([128, LLOC], f32, tag=f"y2_{a}", name=f"y2_{a}")
                     for a in range(HT)]

            # ---------- pipeline ----------
            with (
                tc.tile_pool(name="ps_fwd", bufs=2, space="PSUM") as ps_fwd,
                tc.tile_pool(name="ps_y1", bufs=1, space="PSUM") as ps_y1,
                tc.tile_pool(name="ps_lin", bufs=2, space="PSUM") as ps_lin,
            ):
                # PE clock warmup: one long accumulation group on a tiny
                # early-landing tile (no inter-op waits) ramps the clock
                # while real inputs stream in
                wm_ps = ps_y1.tile([128, HT, F], f32, tag="y1ps", name="wm_ps")
                NWARM = 10
                for w in range(NWARM):
                    nc.tensor.matmul(wm_ps[:, 0, 0:128], lhsT=warm_sb[:, 0:128].bitcast(f32r),
                                     rhs=warm_sb[:, 0:128].bitcast(f32r),
                                     start=(w == 0), stop=(w == NWARM - 1))
                wm_out = scratch.tile([128, 1], f32, tag="wmout")
                nc.vector.tensor_copy(out=wm_out, in_=wm_ps[:, 0, 0:1])

                def emit_kgen():
                    # khat in bf16; psum borrowed from the linear pool (its
                    # real users run much later)
                    kr_ps = ps_lin.tile([128, H], f32, tag="y2ps", name="kr_ps")
                    ki_ps = ps_lin.tile([128, H], f32, tag="y2ps", name="ki_ps")
                    nc.tensor.matmul(kr_ps, lhsT=p1_sb, rhs=wrt_sb, start=True, stop=False)
                    nc.tensor.matmul(kr_ps, lhsT=p2_sb, rhs=wit_sb, start=False, stop=True)
                    nc.tensor.matmul(ki_ps, lhsT=p3_sb, rhs=wrt_sb, start=True, stop=False)
                    nc.tensor.matmul(ki_ps, lhsT=p4_sb, rhs=wit_sb, start=False, stop=True)
                    nc.vector.tensor_copy(out=krb_sb, in_=kr_ps)
                    nc.vector.tensor_copy(out=kib_sb, in_=ki_ps)

                fwd_tiles = {}

                def emit_fwd(c):
                    if c % 2 == 0:
                        src, q0 = u_sb, 3 * c // 2
                    else:
                        src, q0 = u2_sb, (3 * c - 1) // 2
                    ur_ps = ps_fwd.tile([128, H], f32, tag="ur", name=f"ur_{c}")
                    ui_ps = ps_fwd.tile([128, H], f32, tag="ui", name=f"ui_{c}")
                    for a in range(2):
                        rhs = src[:, q0 + a, :]
                        nc.tensor.matmul(ur_ps, lhsT=dfc_sb[:, a, :], rhs=rhs,
                                         start=(a == 0), stop=(a == 1))
                        nc.tensor.matmul(ui_ps, lhsT=dfsn_sb[:, a, :], rhs=rhs,
                                         start=(a == 0), stop=(a == 1))
                    fwd_tiles[c] = (ur_ps, ui_ps)

                def emit_tail(c):
                    ur_ps, ui_ps = fwd_tiles.pop(c)
                    urb = scratch.tile([128, H], bf16, tag="urb", name=f"urb_{c}")
                    uib = scratch.tile([128, H], bf16, tag="uib", name=f"uib_{c}")
                    nc.vector.tensor_copy(out=urb, in_=ur_ps)
                    nc.vector.tensor_copy(out=uib, in_=ui_ps)
                    m1 = scratch.tile([128, H], bf16, tag="m1", name=f"m1_{c}")
                    m2 = scratch.tile([128, H], bf16, tag="m2", name=f"m2_{c}")
                    m3 = scratch.tile([128, H], bf16, tag="m3", name=f"m3_{c}")
                    m4 = scratch.tile([128, H], bf16, tag="m4", name=f"m4_{c}")
                    pr = scratch.tile([128, H], bf16, tag="pr", name=f"pr_{c}")
                    pi = scratch.tile([128, H], bf16, tag="pi", name=f"pi_{c}")
                    nc.vector.tensor_mul(m1, urb, krb_sb)
                    nc.vector.tensor_mul(m2, uib, kib_sb)
                    nc.vector.tensor_mul(m3, urb, kib_sb)
                    nc.vector.tensor_mul(m4, uib, krb_sb)
                    hh = H // 2
                    nc.gpsimd.tensor_sub(pr[:, :hh], m1[:, :hh], m2[:, :hh])
                    nc.vector.tensor_sub(pr[:, hh:], m1[:, hh:], m2[:, hh:])
                    nc.gpsimd.tensor_add(pi[:, :hh], m3[:, :hh], m4[:, :hh])
                    nc.vector.tensor_add(pi[:, hh:], m3[:, hh:], m4[:, hh:])

                    nt = min(HOP, LLOC - c * HOP)
                    y1_ps = ps_y1.tile([128, HT, F], f32, tag="y1ps", name=f"y1ps_{c}")
                    for a in range(HT):
                        nc.tensor.matmul(y1_ps[:, a, :nt],
                                         lhsT=pr[:, a * 128:(a + 1) * 128],
                                         rhs=iccb_sb[:, HALO:HALO + nt],
                                         start=True, stop=False)
                        nc.tensor.matmul(y1_ps[:, a, :nt],
                                         lhsT=pi[:, a * 128:(a + 1) * 128],
                                         rhs=icsnb_sb[:, HALO:HALO + nt],
                                         start=False, stop=True)
                    nc.scalar.activation(out=y1_sb[:, :, c * HOP:c * HOP + nt],
                                         in_=y1_ps[:, :, :nt], func=GELU)

                def do_linear_half(lc2):
                    for ao in range(HT):
                        y2_ps = ps_lin.tile([128, 512], f32, tag="y2ps",
                                            name=f"y2ps_{lc2}_{ao}")
                        for ai in range(HT):
                            nc.tensor.matmul(
                                y2_ps,
                                lhsT=lwt_sb[:, ai, ao * 128:(ao + 1) * 128],
                                rhs=y1_sb[:, ai, lc2 * 512:(lc2 + 1) * 512],
                                start=(ai == 0), stop=(ai == HT - 1))
                        nc.scalar.activation(out=y2_sb[ao][:, lc2 * 512:(lc2 + 1) * 512],
                                             in_=y2_ps, func=GELU,
                                             bias=lb_sb[:, ao:ao + 1])
                        eng = nc.sync if ao % 2 == 0 else nc.scalar
                        eng.dma_start(
                            out=y2[ao * 128:(ao + 1) * 128, lc2 * 512:(lc2 + 1) * 512],
                            in_=y2_sb[ao][:, lc2 * 512:(lc2 + 1) * 512])

                # software-pipelined emission: window c's forward goes into
                # the PE stream before window c-1's chain-dependent inverse, so
                # the in-order PE never head-of-line blocks on the DVE/ACT
                # chain; kgen is emitted after fwd0/fwd1 (it gates only the
                # first product)
                emit_kgen()
                for c in range(NWIN):
                    emit_fwd(c)
                    if c >= 1:
                        emit_tail(c - 1)
                    if c == 3:
                        do_linear_half(0)   # y1 cols [0,512) done (w0-2)
                emit_tail(NWIN - 1)
                do_linear_half(1)

    nc.compile()
    return nc


def _build_tables(frequencies, decays, W, lin_w, lin_b):
    lam_re = (-np.exp(decays.astype(np.float32))).astype(np.float32)
    m = np.arange(K, dtype=np.float32)
    # match the reference's fp32 rounding of Lam[:,None] * t
    re = (lam_re[:, None] * m[None, :]).astype(np.float32)
    im = (frequencies.astype(np.float32)[:, None] * m[None, :]).astype(np.float32)
    mag = np.exp(re.astype(np.float64))
    sc = (mag * np.cos(im.astype(np.float64))).astype(np.float32)
    ssn = (-mag * np.sin(im.astype(np.float64))).astype(np.float32)

    fb = np.arange(F // 2, dtype=np.float64) + 0.5
    tt = np.arange(F, dtype=np.float64)
    ang = 2 * np.pi * np.outer(tt, fb) / F
    dfc = np.cos(ang).astype(np.float32)
    dfsn = (-np.sin(ang)).astype(np.float32)
    iang = 2 * np.pi * np.outer(fb, tt) / F
    icc = ((2.0 / F) * np.cos(iang)).astype(np.float32)
    icsn = ((-2.0 / F) * np.sin(iang)).astype(np.float32)

    def to_bf16_bits(x):
        u = x.astype(np.float32).view(np.uint32)
        r = (u + 0x7FFF + ((u >> 16) & 1)) >> 16
        return r.astype(np.uint16)

    blob_a = np.zeros((64, 1536), np.float32)
    blob_a[:, 0:512] = W[..., 0].T.astype(np.float32)
    blob_a[:, 512:1024] = W[..., 1].T.astype(np.float32)
    # P tables fold the Vandermonde and the forward DFT of the kernel:
    # khat_r = P1^T @ WrT + P2^T @ WiT ; khat_i = P3^T @ WrT + P4^T @ WiT
    sc64 = sc.astype(np.float64); ssn64 = ssn.astype(np.float64)
    dfc64 = dfc[:K].astype(np.float64); dfsn64 = dfsn[:K].astype(np.float64)
    blob_a[:, 1024:1152] = (sc64 @ dfc64).astype(np.float32)
    blob_a[:, 1152:1280] = (ssn64 @ dfc64).astype(np.float32)
    blob_a[:, 1280:1408] = (sc64 @ dfsn64).astype(np.float32)
    blob_a[:, 1408:1536] = (ssn64 @ dfsn64).astype(np.float32)
    blob_b = np.zeros((128, 516), np.float32)
    blob_b[:, 0:128] = dfc[0:128]
    blob_b[:, 128:256] = dfc[128:256]
    blob_b[:, 256:384] = dfsn[0:128]
    blob_b[:, 384:512] = dfsn[128:256]
    blob_b[:, 512:516] = lin_b.astype(np.float32).reshape(4, 128).T
    blob_c = np.zeros((128, 512), np.uint16)
    blob_c[:, 0:256] = to_bf16_bits(icc)
    blob_c[:, 256:512] = to_bf16_bits(icsn)
    return {
        "bloba": np.ascontiguousarray(blob_a),
        "blobb": np.ascontiguousarray(blob_b),
        "blobc": np.ascontiguousarray(blob_c),
        "lwt": np.ascontiguousarray(to_bf16_bits(lin_w.astype(np.float32).T)),
    }


def kernel(u, frequencies, decays, W, lin_w, lin_b):
    from concourse.bass_utils import run_bass_kernel_spmd

    u = np.asarray(u, dtype=np.float32)
    tables = _build_tables(np.asarray(frequencies), np.asarray(decays),
                           np.asarray(W), np.asarray(lin_w), np.asarray(lin_b))

    if "nc" not in _cache:
        _cache["nc"] = _build_nc()
    nc = _cache["nc"]

    in_maps = []
    for b in range(B):
        for half in range(2):
            lo = half * LLOC
            uT = np.zeros((RPAD, H), np.float32)
            a0 = lo - HALO
            s0 = max(a0, 0)
            s1 = min(a0 + ROWS, L)
            uT[s0 - a0:s1 - a0] = u[b, :, s0:s1].T
            in_maps.append({"ut": np.ascontiguousarray(uT), **tables})

    res = run_bass_kernel_spmd(nc, in_maps, core_ids=list(range(NCORES)))
    out = np.empty((B, H, L), np.float32)
    for i, r in enumerate(res.results):
        b, half = divmod(i, 2)
        out[b, :, half * LLOC:(half + 1) * LLOC] = r["y2"]
    return out

